# revision 3
# baseline (speedup 1.0000x reference)
"""Trainium2 Bass kernel for nn_Decoder_17841294148315.

Computation (full shapes):
    key_vecT [32,512] = (gelu(raw_Z.T @ W1z + b1z) @ W2z + b2z).T   (all transposed-space)
    queryT   [32,16384] from G_rep MLP
    scores   [16384,512] = (query @ key_vec.T) / sqrt(32)
    p_attn   = softmax(scores + gumbel(key 42), axis=-1)
    out      = (p_attn @ gen_Z.T).T            [16384 cells, 16384 genes] fp32

Sharding: genes (rows of G_rep / scores / p_attn, cols of out) split across
8 NeuronCores; every core computes the (small) key MLP redundantly.
Matmuls run in fp32r (TF32-like, 1 PE cycle/row at free>=256).
"""

import sys

sys.path.insert(0, "/opt/trn_rl_repo")

import numpy as np

import concourse.bass as bass
import concourse.mybir as mybir
import concourse.tile as tile_mod
from concourse.bass_utils import run_bass_kernel_spmd
from concourse.masks import make_identity
from concourse.tile import TileContext
from concourse.vector_clock import ScopedClock

N_CORES = 8
G, Zc, D, Rg, KD, H = 16384, 16384, 512, 128, 32, 256
GS = G // N_CORES  # genes per core
F32 = mybir.dt.float32
F32R = mybir.dt.float32r
SCALE = 1.0 / np.sqrt(np.float32(KD))  # folded into key_vecT

# ---------------------------------------------------------------------------
# Workarounds for this container's walrus build, which rejects >1 sync-wait
# per instruction ("Too many sync wait commands").


def _patched_drain_and_barrier(self, tick_clock, wait_clock):
    nc = self.nc
    drain_inst = nc.sync.drain()
    wait_clock.add_sem_waits(
        drain_inst.ins, ScopedClock({None: tick_clock.global_clock})
    )
    si = drain_inst.ins.sync_info
    waits = list(si.on_wait or [])
    if len(waits) > 1:
        si.on_wait = waits[:1]
        for w in waits[1:]:
            nop = nc.sync.nop(nofuse=True)
            nop.ins.sync_info = mybir.SyncInfo(on_wait=[w], on_update=[])
    nc.all_engine_barrier()
    popped = nc._tile_sem_poison_stack.pop()
    assert popped is self._sem_poison
    nc.clear_and_free_semaphores(list(self.sems.allocated().values()))
    nc.all_engine_barrier()


tile_mod.TileContext._drain_and_barrier = _patched_drain_and_barrier

_split_uid = [0]


def split_multi_waits(nc, max_waits=1):
    """Move excess per-instruction sync-waits onto same-engine NOPs."""
    for f in nc.m.functions:
        for blk in f.blocks:
            new_insts = []
            for inst in blk.instructions:
                si = inst.sync_info
                if si is not None and si.on_wait and len(si.on_wait) > max_waits:
                    waits = list(si.on_wait)
                    keep = waits[-max_waits:]
                    extra = waits[:-max_waits]
                    for i in range(0, len(extra), max_waits):
                        _split_uid[0] += 1
                        nop = mybir.InstNoOp(
                            name=f"WSPLIT-{_split_uid[0]}",
                            engine=inst.engine,
                            bass_nofuse=True,
                            sync_info=mybir.SyncInfo(
                                on_wait=extra[i : i + max_waits], on_update=[]
                            ),
                        )
                        new_insts.append(nop)
                    si.on_wait = keep
                new_insts.append(inst)
            blk.instructions[:] = new_insts


# ---------------------------------------------------------------------------


def build_nc():
    nc = bass.Bass("TRN2", target_bir_lowering=False, debug=False, num_devices=N_CORES)

    raw_Z = nc.dram_tensor("raw_Z", [Zc, D], F32, kind="ExternalInput")
    W1z = nc.dram_tensor("W1z", [Zc, H], F32, kind="ExternalInput")
    b1z = nc.dram_tensor("b1z", [H], F32, kind="ExternalInput")
    W2z = nc.dram_tensor("W2z", [H, KD], F32, kind="ExternalInput")
    b2z = nc.dram_tensor("b2z", [KD], F32, kind="ExternalInput")
    W1g = nc.dram_tensor("W1g", [Rg, KD], F32, kind="ExternalInput")
    b1g = nc.dram_tensor("b1g", [KD], F32, kind="ExternalInput")
    W2g = nc.dram_tensor("W2g", [KD, KD], F32, kind="ExternalInput")
    b2g = nc.dram_tensor("b2g", [KD], F32, kind="ExternalInput")
    G_rep_s = nc.dram_tensor("G_rep_s", [GS, Rg], F32, kind="ExternalInput")
    gumb_s = nc.dram_tensor("gumb_s", [GS, D], F32, kind="ExternalInput")
    gen_Z = nc.dram_tensor("gen_Z", [Zc, D], F32, kind="ExternalInput")
    out_s = nc.dram_tensor("out_s", [Zc, GS], F32, kind="ExternalOutput")

    NT = GS // 128  # gene tiles per core (16)
    KT = D // 128  # z k-tiles (4)
    NB = GS // 512  # output n-blocks per m-tile (4)

    with TileContext(nc) as tc:
        with (
            tc.tile_pool(name="singles", bufs=1) as singles,
            tc.tile_pool(name="s_w1z", bufs=3) as s_w1z,
            tc.tile_pool(name="s_rawz", bufs=3) as s_rawz,
            tc.tile_pool(name="s_grep", bufs=2) as s_grep,
            tc.tile_pool(name="s_gumb", bufs=2) as s_gumb,
            tc.tile_pool(name="s_genz", bufs=3) as s_genz,
            tc.tile_pool(name="s_lhst", bufs=3) as s_lhst,
            tc.tile_pool(name="s_out", bufs=3) as s_out,
            tc.tile_pool(name="s_tmp", bufs=3) as s_tmp,
            tc.tile_pool(name="s_small", bufs=4) as s_small,
            tc.tile_pool(name="ps_acc", bufs=2, space="PSUM") as ps_acc,
            tc.tile_pool(name="ps_mm", bufs=4, space="PSUM") as ps_mm,
            tc.tile_pool(name="ps_tr", bufs=2, space="PSUM") as ps_tr,
        ):
            ident = singles.tile([128, 128], F32)
            make_identity(nc, ident)

            # --- constants / small weights ---
            w2z_sb = singles.tile([128, 2, KD], F32R)
            nc.sync.dma_start(
                w2z_sb[:], W2z[:].rearrange("(o p) k -> p o k", p=128).bitcast(F32R)
            )
            b1z_sb = singles.tile([128, 2], F32)
            nc.sync.dma_start(b1z_sb[:], b1z[:].rearrange("(o p) -> p o", p=128))
            b2z_sb = singles.tile([KD, 1], F32)
            nc.sync.dma_start(b2z_sb[:], b2z[:, None])
            w1g_sb = singles.tile([Rg, KD], F32R)
            nc.sync.dma_start(w1g_sb[:], W1g[:].bitcast(F32R))
            b1g_sb = singles.tile([KD, 1], F32)
            nc.sync.dma_start(b1g_sb[:], b1g[:, None])
            w2g_sb = singles.tile([KD, KD], F32R)
            nc.sync.dma_start(w2g_sb[:], W2g[:].bitcast(F32R))
            b2g_sb = singles.tile([KD, 1], F32)
            nc.sync.dma_start(b2g_sb[:], b2g[:, None])

            # --- phase A: key MLP (replicated) ---
            # hT [256,512] = (raw_Z.T @ W1z).T accumulated over 128 k-tiles
            psum_h = [
                ps_acc.tile([128, 512], F32, tag="acc", name=f"psum_h{i}")
                for i in range(2)
            ]
            NKA = Zc // 128
            for k in range(NKA):
                w1z_t = s_w1z.tile([128, H], F32R)
                nc.sync.dma_start(
                    w1z_t[:], W1z[k * 128 : (k + 1) * 128, :].bitcast(F32R)
                )
                rawz_t = s_rawz.tile([128, D], F32R)
                nc.sync.dma_start(
                    rawz_t[:], raw_Z[k * 128 : (k + 1) * 128, :].bitcast(F32R)
                )
                for hp in range(2):
                    nc.tensor.matmul(
                        psum_h[hp][:],
                        w1z_t[:, hp * 128 : (hp + 1) * 128],
                        rawz_t[:],
                        start=(k == 0),
                        stop=(k == NKA - 1),
                    )
            hT_sb = singles.tile([128, 2, D], F32R)
            for hp in range(2):
                nc.scalar.activation(
                    hT_sb[:, hp, :],
                    psum_h[hp][:],
                    mybir.ActivationFunctionType.Gelu,
                    bias=b1z_sb[:, hp : hp + 1],
                )
            # key_vecT [32,512] = W2z.T @ hT ; fold bias and 1/sqrt(32)
            psum_kv = ps_mm.tile([KD, 512], F32, tag="mm")
            for hp in range(2):
                nc.tensor.matmul(
                    psum_kv[:],
                    w2z_sb[:, hp, :],
                    hT_sb[:, hp, :],
                    start=(hp == 0),
                    stop=(hp == 1),
                )
            kvT_sb = singles.tile([KD, 512], F32R)
            nc.vector.tensor_scalar(
                kvT_sb[:],
                psum_kv[:],
                b2z_sb[:],
                float(SCALE),
                mybir.AluOpType.add,
                mybir.AluOpType.mult,
            )

            # --- phase B: query path for this core's gene slice ---
            grepT_sb = singles.tile([128, NT, 128], F32R)
            for t in range(NT):
                g_t = s_grep.tile([128, Rg], F32)
                nc.sync.dma_start(g_t[:], G_rep_s[t * 128 : (t + 1) * 128, :])
                psum_gt = ps_tr.tile([128, 512], F32, tag="tr")
                nc.tensor.transpose(psum_gt[:, 0:128], g_t[:], ident[:])
                nc.vector.tensor_copy(grepT_sb[:, t, :], psum_gt[:, 0:128])
            q1T_sb = singles.tile([KD, GS], F32R)
            for nb in range(NB):
                psum_q1 = ps_mm.tile([KD, 512], F32, tag="mm")
                nc.tensor.matmul(
                    psum_q1[:],
                    w1g_sb[:],
                    grepT_sb[:, 4 * nb : 4 * nb + 4, :],
                    start=True,
                    stop=True,
                )
                nc.scalar.activation(
                    q1T_sb[:, nb * 512 : (nb + 1) * 512],
                    psum_q1[:],
                    mybir.ActivationFunctionType.Gelu,
                    bias=b1g_sb[:],
                )
            qT_sb = singles.tile([KD, GS], F32R)
            for nb in range(NB):
                psum_q2 = ps_mm.tile([KD, 512], F32, tag="mm")
                nc.tensor.matmul(
                    psum_q2[:],
                    w2g_sb[:],
                    q1T_sb[:, nb * 512 : (nb + 1) * 512],
                    start=True,
                    stop=True,
                )
                nc.vector.tensor_scalar(
                    qT_sb[:, nb * 512 : (nb + 1) * 512],
                    psum_q2[:],
                    b2g_sb[:],
                    None,
                    mybir.AluOpType.add,
                )

            # --- phase C: scores + gumbel-softmax + transpose to pT ---
            pT_sb = singles.tile([128, KT, GS], F32R)
            for t in range(NT):
                psum_s = ps_mm.tile([128, 512], F32, tag="mm")
                nc.tensor.matmul(
                    psum_s[:],
                    qT_sb[:, t * 128 : (t + 1) * 128],
                    kvT_sb[:],
                    start=True,
                    stop=True,
                )
                gmb_t = s_gumb.tile([128, D], F32)
                nc.sync.dma_start(gmb_t[:], gumb_s[t * 128 : (t + 1) * 128, :])
                sg_sb = s_tmp.tile([128, D], F32, tag="sg")
                nc.vector.tensor_add(sg_sb[:], psum_s[:], gmb_t[:])
                e_sb = s_tmp.tile([128, D], F32, tag="e")
                ssum = s_small.tile([128, 1], F32, tag="ssum")
                nc.scalar.activation(
                    e_sb[:],
                    sg_sb[:],
                    mybir.ActivationFunctionType.Exp,
                    accum_out=ssum[:],
                )
                rin = s_small.tile([128, 1], F32, tag="rin")
                nc.vector.reciprocal(rin[:], ssum[:])
                p_sb = s_tmp.tile([128, D], F32, tag="p")
                nc.vector.tensor_scalar_mul(p_sb[:], e_sb[:], rin[:])
                psum_pt = ps_tr.tile([128, 512], F32, tag="tr")
                for j in range(KT):
                    nc.tensor.transpose(
                        psum_pt[:, j * 128 : (j + 1) * 128],
                        p_sb[:, j * 128 : (j + 1) * 128],
                        ident[:],
                    )
                nc.vector.tensor_copy(
                    pT_sb[:, :, t * 128 : (t + 1) * 128],
                    psum_pt[:].rearrange("p (j c) -> p j c", j=KT),
                )

            # --- phase D: out[cells, genes] = gen_Z @ p_attn.T ---
            NM = Zc // 128
            for m in range(NM):
                gz_t = s_genz.tile([128, D], F32)
                nc.sync.dma_start(gz_t[:], gen_Z[m * 128 : (m + 1) * 128, :])
                psum_gz = ps_tr.tile([128, 512], F32, tag="tr")
                for j in range(KT):
                    nc.tensor.transpose(
                        psum_gz[:, j * 128 : (j + 1) * 128],
                        gz_t[:, j * 128 : (j + 1) * 128],
                        ident[:],
                    )
                lhsT_t = s_lhst.tile([128, KT, 128], F32R)
                nc.vector.tensor_copy(
                    lhsT_t[:], psum_gz[:].rearrange("p (j c) -> p j c", j=KT)
                )
                out_sb = s_out.tile([128, GS], F32)
                for n in range(NB):
                    psum_o = ps_mm.tile([128, 512], F32, tag="mm")
                    for k in range(KT):
                        nc.tensor.matmul(
                            psum_o[:],
                            lhsT_t[:, k, :],
                            pT_sb[:, k, n * 512 : (n + 1) * 512],
                            start=(k == 0),
                            stop=(k == KT - 1),
                        )
                    nc.vector.tensor_copy(out_sb[:, n * 512 : (n + 1) * 512], psum_o[:])
                nc.sync.dma_start(out_s[m * 128 : (m + 1) * 128, :], out_sb[:])

    split_multi_waits(nc)
    return nc


_NC_CACHE = {}
LAST_RESULTS = None


def _gumbel_full():
    import jax

    cpu = jax.devices("cpu")[0]
    with jax.default_device(cpu):
        g = jax.random.gumbel(jax.random.key(42), (G, D), "float32")
        return np.asarray(g)


def kernel(
    raw_Z, gen_Z, W1z, b1z, W2z, b2z, W1g, b1g, W2g, b2g, G_rep, **kw
) -> np.ndarray:
    global LAST_RESULTS
    if "nc" not in _NC_CACHE:
        _NC_CACHE["nc"] = build_nc()
    nc = _NC_CACHE["nc"]

    gumb = _gumbel_full()
    shared = {
        "raw_Z": np.ascontiguousarray(np.asarray(raw_Z, np.float32)),
        "W1z": np.ascontiguousarray(np.asarray(W1z, np.float32)),
        "b1z": np.asarray(b1z, np.float32),
        "W2z": np.ascontiguousarray(np.asarray(W2z, np.float32)),
        "b2z": np.asarray(b2z, np.float32),
        "W1g": np.ascontiguousarray(np.asarray(W1g, np.float32)),
        "b1g": np.asarray(b1g, np.float32),
        "W2g": np.ascontiguousarray(np.asarray(W2g, np.float32)),
        "b2g": np.asarray(b2g, np.float32),
        "gen_Z": np.ascontiguousarray(np.asarray(gen_Z, np.float32)),
    }
    G_rep = np.asarray(G_rep, np.float32)
    in_maps = []
    for c in range(N_CORES):
        m = dict(shared)
        m["G_rep_s"] = np.ascontiguousarray(G_rep[c * GS : (c + 1) * GS])
        m["gumb_s"] = np.ascontiguousarray(gumb[c * GS : (c + 1) * GS])
        in_maps.append(m)

    res = run_bass_kernel_spmd(nc, in_maps, core_ids=list(range(N_CORES)))
    LAST_RESULTS = res
    out = np.concatenate([res.results[c]["out_s"] for c in range(N_CORES)], axis=1)
    return out


# revision 4
# speedup vs baseline: 1.1718x; 1.1718x over previous
"""Trainium2 Bass kernel for nn_Decoder_17841294148315.

Computation (full shapes):
    key_vecT [32,512] = (gelu(raw_Z.T @ W1z + b1z) @ W2z + b2z).T
    queryT   [32,16384] from G_rep MLP
    scores   [16384,512] = (query @ key_vec.T) / sqrt(32)
    p_attn   = softmax(scores + gumbel(key 42), axis=-1)
    out      = (p_attn @ gen_Z.T).T          [16384 cells, 16384 genes] fp32

Sharding: genes split across 8 NeuronCores. The key MLP's first matmul
contracts over cells; it is sharded over cells with an AllReduce of the
[256,512] pre-gelu activations. The big output matmul runs in fp16
(inputs rounded on the PSUM->SBUF copies), everything else in
fp32r (TF32-like).
"""

import sys

sys.path.insert(0, "/opt/trn_rl_repo")

import numpy as np

import concourse.bass as bass
import concourse.mybir as mybir
import concourse.tile as tile_mod
from concourse.bass_utils import run_bass_kernel_spmd
from concourse.masks import make_identity
from concourse.tile import TileContext
from concourse.vector_clock import ScopedClock

N_CORES = 8
G, Zc, D, Rg, KD, H = 16384, 16384, 512, 128, 32, 256
GS = G // N_CORES  # genes per core
CS = Zc // N_CORES  # cells per core (key-MLP contraction shard)
F32 = mybir.dt.float32
F32R = mybir.dt.float32r
F16 = mybir.dt.float16
SCALE = 1.0 / np.sqrt(np.float32(KD))  # folded into key_vecT

SHARD_A = True  # shard the key MLP over cells + AllReduce
MAIN_DT = F16  # dtype of the big output matmul

# ---------------------------------------------------------------------------
# Workarounds for this container's walrus build, which rejects >1 sync-wait
# per instruction ("Too many sync wait commands").


def _patched_drain_and_barrier(self, tick_clock, wait_clock):
    nc = self.nc
    drain_inst = nc.sync.drain()
    wait_clock.add_sem_waits(
        drain_inst.ins, ScopedClock({None: tick_clock.global_clock})
    )
    si = drain_inst.ins.sync_info
    waits = list(si.on_wait or [])
    if len(waits) > 1:
        si.on_wait = waits[:1]
        for w in waits[1:]:
            nop = nc.sync.nop(nofuse=True)
            nop.ins.sync_info = mybir.SyncInfo(on_wait=[w], on_update=[])
    nc.all_engine_barrier()
    popped = nc._tile_sem_poison_stack.pop()
    assert popped is self._sem_poison
    nc.clear_and_free_semaphores(list(self.sems.allocated().values()))
    nc.all_engine_barrier()


tile_mod.TileContext._drain_and_barrier = _patched_drain_and_barrier

_split_uid = [0]


def split_multi_waits(nc, max_waits=1):
    """Move excess per-instruction sync-waits onto same-engine NOPs."""
    for f in nc.m.functions:
        for blk in f.blocks:
            new_insts = []
            for inst in blk.instructions:
                si = inst.sync_info
                if si is not None and si.on_wait and len(si.on_wait) > max_waits:
                    waits = list(si.on_wait)
                    keep = waits[-max_waits:]
                    extra = waits[:-max_waits]
                    for i in range(0, len(extra), max_waits):
                        _split_uid[0] += 1
                        nop = mybir.InstNoOp(
                            name=f"WSPLIT-{_split_uid[0]}",
                            engine=inst.engine,
                            bass_nofuse=True,
                            sync_info=mybir.SyncInfo(
                                on_wait=extra[i : i + max_waits], on_update=[]
                            ),
                        )
                        new_insts.append(nop)
                    si.on_wait = keep
                new_insts.append(inst)
            blk.instructions[:] = new_insts


# ---------------------------------------------------------------------------


def build_nc():
    nc = bass.Bass("TRN2", target_bir_lowering=False, debug=False, num_devices=N_CORES)

    if SHARD_A:
        raw_Z_s = nc.dram_tensor("raw_Z_s", [CS, D], F32, kind="ExternalInput")
        W1z_s = nc.dram_tensor("W1z_s", [CS, H], F32, kind="ExternalInput")
    else:
        raw_Z_s = nc.dram_tensor("raw_Z_s", [Zc, D], F32, kind="ExternalInput")
        W1z_s = nc.dram_tensor("W1z_s", [Zc, H], F32, kind="ExternalInput")
    b1z = nc.dram_tensor("b1z", [H], F32, kind="ExternalInput")
    W2z = nc.dram_tensor("W2z", [H, KD], F32, kind="ExternalInput")
    b2z = nc.dram_tensor("b2z", [KD], F32, kind="ExternalInput")
    W1g = nc.dram_tensor("W1g", [Rg, KD], F32, kind="ExternalInput")
    b1g = nc.dram_tensor("b1g", [KD], F32, kind="ExternalInput")
    W2g = nc.dram_tensor("W2g", [KD, KD], F32, kind="ExternalInput")
    b2g = nc.dram_tensor("b2g", [KD], F32, kind="ExternalInput")
    G_rep_s = nc.dram_tensor("G_rep_s", [GS, Rg], F32, kind="ExternalInput")
    gumb_s = nc.dram_tensor("gumb_s", [GS, D], F32, kind="ExternalInput")
    gen_Z = nc.dram_tensor("gen_Z", [Zc, D], F32, kind="ExternalInput")
    out_s = nc.dram_tensor("out_s", [Zc, GS], F32, kind="ExternalOutput")

    NT = GS // 128  # gene tiles per core (16)
    KT = D // 128  # z k-tiles (4)
    NB = GS // 512  # output n-blocks per m-tile (4)

    with TileContext(nc) as tc:
        with (
            tc.tile_pool(name="singles", bufs=1) as singles,
            tc.tile_pool(name="s_w1z", bufs=3) as s_w1z,
            tc.tile_pool(name="s_rawz", bufs=3) as s_rawz,
            tc.tile_pool(name="s_grep", bufs=2) as s_grep,
            tc.tile_pool(name="s_gumb", bufs=2) as s_gumb,
            tc.tile_pool(name="s_genz", bufs=3) as s_genz,
            tc.tile_pool(name="s_lhst", bufs=3) as s_lhst,
            tc.tile_pool(name="s_out", bufs=3) as s_out,
            tc.tile_pool(name="s_tmp", bufs=3) as s_tmp,
            tc.tile_pool(name="s_small", bufs=4) as s_small,
            tc.tile_pool(name="dram", bufs=1, space="DRAM") as dram,
            tc.tile_pool(name="ps_acc", bufs=2, space="PSUM") as ps_acc,
            tc.tile_pool(name="ps_mm", bufs=4, space="PSUM") as ps_mm,
            tc.tile_pool(name="ps_tr", bufs=2, space="PSUM") as ps_tr,
        ):
            ident = singles.tile([128, 128], F32)
            make_identity(nc, ident)

            # --- constants / small weights ---
            w2z_sb = singles.tile([128, 2, KD], F32R)
            nc.sync.dma_start(
                w2z_sb[:], W2z[:].rearrange("(o p) k -> p o k", p=128).bitcast(F32R)
            )
            b1z_sb = singles.tile([128, 2], F32)
            nc.sync.dma_start(b1z_sb[:], b1z[:].rearrange("(o p) -> p o", p=128))
            b2z_sb = singles.tile([KD, 1], F32)
            nc.sync.dma_start(b2z_sb[:], b2z[:, None])
            w1g_sb = singles.tile([Rg, KD], F32R)
            nc.sync.dma_start(w1g_sb[:], W1g[:].bitcast(F32R))
            b1g_sb = singles.tile([KD, 1], F32)
            nc.sync.dma_start(b1g_sb[:], b1g[:, None])
            w2g_sb = singles.tile([KD, KD], F32R)
            nc.sync.dma_start(w2g_sb[:], W2g[:].bitcast(F32R))
            b2g_sb = singles.tile([KD, 1], F32)
            nc.sync.dma_start(b2g_sb[:], b2g[:, None])

            # --- phase A: key MLP first layer ---
            # hT [256,512] = (raw_Z.T @ W1z).T ; contraction over cells
            psum_h = [
                ps_acc.tile([128, 512], F32, tag="acc", name=f"psum_h{i}")
                for i in range(2)
            ]
            NKA = (CS if SHARD_A else Zc) // 128
            for k in range(NKA):
                w1z_t = s_w1z.tile([128, H], F32R)
                nc.sync.dma_start(
                    w1z_t[:], W1z_s[k * 128 : (k + 1) * 128, :].bitcast(F32R)
                )
                rawz_t = s_rawz.tile([128, D], F32R)
                nc.sync.dma_start(
                    rawz_t[:], raw_Z_s[k * 128 : (k + 1) * 128, :].bitcast(F32R)
                )
                for hp in range(2):
                    nc.tensor.matmul(
                        psum_h[hp][:],
                        w1z_t[:, hp * 128 : (hp + 1) * 128],
                        rawz_t[:],
                        start=(k == 0),
                        stop=(k == NKA - 1),
                    )
            hT_sb = singles.tile([128, 2, D], F32R)
            if SHARD_A:
                hpart_sb = singles.tile([128, 2, D], F32)
                for hp in range(2):
                    nc.vector.tensor_copy(hpart_sb[:, hp, :], psum_h[hp][:])
                h_in_b = dram.tile([128, 2 * D], F32)
                h_out_b = dram.tile([128, 2 * D], F32)
                nc.gpsimd.dma_start(
                    h_in_b[:], hpart_sb[:].rearrange("p a b -> p (a b)")
                )
                nc.gpsimd.collective_compute(
                    "AllReduce",
                    mybir.AluOpType.add,
                    replica_groups=[list(range(N_CORES))],
                    ins=[h_in_b.opt()],
                    outs=[h_out_b.opt()],
                )
                hsum_sb = singles.tile([128, 2, D], F32)
                nc.gpsimd.dma_start(
                    hsum_sb[:], h_out_b[:].rearrange("p (a b) -> p a b", a=2)
                )
                for hp in range(2):
                    nc.scalar.activation(
                        hT_sb[:, hp, :],
                        hsum_sb[:, hp, :],
                        mybir.ActivationFunctionType.Gelu,
                        bias=b1z_sb[:, hp : hp + 1],
                    )
            else:
                for hp in range(2):
                    nc.scalar.activation(
                        hT_sb[:, hp, :],
                        psum_h[hp][:],
                        mybir.ActivationFunctionType.Gelu,
                        bias=b1z_sb[:, hp : hp + 1],
                    )
            # key_vecT [32,512] = W2z.T @ hT ; fold bias and 1/sqrt(32)
            psum_kv = ps_mm.tile([KD, 512], F32, tag="mm")
            for hp in range(2):
                nc.tensor.matmul(
                    psum_kv[:],
                    w2z_sb[:, hp, :],
                    hT_sb[:, hp, :],
                    start=(hp == 0),
                    stop=(hp == 1),
                )
            kvT_sb = singles.tile([KD, 512], F32R)
            nc.vector.tensor_scalar(
                kvT_sb[:],
                psum_kv[:],
                b2z_sb[:],
                float(SCALE),
                mybir.AluOpType.add,
                mybir.AluOpType.mult,
            )

            # --- phase B: query path for this core's gene slice ---
            grepT_sb = singles.tile([128, NT, 128], F32R)
            for t in range(NT):
                g_t = s_grep.tile([128, Rg], F32)
                nc.sync.dma_start(g_t[:], G_rep_s[t * 128 : (t + 1) * 128, :])
                psum_gt = ps_tr.tile([128, 512], F32, tag="tr")
                nc.tensor.transpose(psum_gt[:, 0:128], g_t[:], ident[:])
                nc.vector.tensor_copy(grepT_sb[:, t, :], psum_gt[:, 0:128])
            q1T_sb = singles.tile([KD, GS], F32R)
            for nb in range(NB):
                psum_q1 = ps_mm.tile([KD, 512], F32, tag="mm")
                nc.tensor.matmul(
                    psum_q1[:],
                    w1g_sb[:],
                    grepT_sb[:, 4 * nb : 4 * nb + 4, :],
                    start=True,
                    stop=True,
                )
                nc.scalar.activation(
                    q1T_sb[:, nb * 512 : (nb + 1) * 512],
                    psum_q1[:],
                    mybir.ActivationFunctionType.Gelu,
                    bias=b1g_sb[:],
                )
            qT_sb = singles.tile([KD, GS], F32R)
            for nb in range(NB):
                psum_q2 = ps_mm.tile([KD, 512], F32, tag="mm")
                nc.tensor.matmul(
                    psum_q2[:],
                    w2g_sb[:],
                    q1T_sb[:, nb * 512 : (nb + 1) * 512],
                    start=True,
                    stop=True,
                )
                nc.vector.tensor_scalar(
                    qT_sb[:, nb * 512 : (nb + 1) * 512],
                    psum_q2[:],
                    b2g_sb[:],
                    None,
                    mybir.AluOpType.add,
                )

            # --- phase C: scores + gumbel-softmax + transpose to pT ---
            pT_sb = singles.tile([128, KT, GS], MAIN_DT)
            for t in range(NT):
                psum_s = ps_mm.tile([128, 512], F32, tag="mm")
                nc.tensor.matmul(
                    psum_s[:],
                    qT_sb[:, t * 128 : (t + 1) * 128],
                    kvT_sb[:],
                    start=True,
                    stop=True,
                )
                gmb_t = s_gumb.tile([128, D], F32)
                nc.sync.dma_start(gmb_t[:], gumb_s[t * 128 : (t + 1) * 128, :])
                sg_sb = s_tmp.tile([128, D], F32, tag="sg")
                nc.vector.tensor_add(sg_sb[:], psum_s[:], gmb_t[:])
                e_sb = s_tmp.tile([128, D], F32, tag="e")
                ssum = s_small.tile([128, 1], F32, tag="ssum")
                nc.scalar.activation(
                    e_sb[:],
                    sg_sb[:],
                    mybir.ActivationFunctionType.Exp,
                    accum_out=ssum[:],
                )
                rin = s_small.tile([128, 1], F32, tag="rin")
                nc.vector.reciprocal(rin[:], ssum[:])
                p_sb = s_tmp.tile([128, D], F32, tag="p")
                nc.vector.tensor_scalar_mul(p_sb[:], e_sb[:], rin[:])
                psum_pt = ps_tr.tile([128, 512], F32, tag="tr")
                for j in range(KT):
                    nc.tensor.transpose(
                        psum_pt[:, j * 128 : (j + 1) * 128],
                        p_sb[:, j * 128 : (j + 1) * 128],
                        ident[:],
                    )
                nc.vector.tensor_copy(
                    pT_sb[:, :, t * 128 : (t + 1) * 128],
                    psum_pt[:].rearrange("p (j c) -> p j c", j=KT),
                )

            # --- phase D: out[cells, genes] = gen_Z @ p_attn.T ---
            NM = Zc // 128
            for m in range(NM):
                gz_t = s_genz.tile([128, D], F32)
                nc.sync.dma_start(gz_t[:], gen_Z[m * 128 : (m + 1) * 128, :])
                psum_gz = ps_tr.tile([128, 512], F32, tag="tr")
                for j in range(KT):
                    nc.tensor.transpose(
                        psum_gz[:, j * 128 : (j + 1) * 128],
                        gz_t[:, j * 128 : (j + 1) * 128],
                        ident[:],
                    )
                lhsT_t = s_lhst.tile([128, KT, 128], MAIN_DT)
                nc.vector.tensor_copy(
                    lhsT_t[:], psum_gz[:].rearrange("p (j c) -> p j c", j=KT)
                )
                out_sb = s_out.tile([128, GS], F32)
                for n in range(NB):
                    psum_o = ps_mm.tile([128, 512], F32, tag="mm")
                    for k in range(KT):
                        nc.tensor.matmul(
                            psum_o[:],
                            lhsT_t[:, k, :],
                            pT_sb[:, k, n * 512 : (n + 1) * 512],
                            start=(k == 0),
                            stop=(k == KT - 1),
                        )
                    if n == NB - 1:
                        nc.scalar.copy(out_sb[:, n * 512 : (n + 1) * 512], psum_o[:])
                    else:
                        nc.vector.tensor_copy(
                            out_sb[:, n * 512 : (n + 1) * 512], psum_o[:]
                        )
                nc.sync.dma_start(out_s[m * 128 : (m + 1) * 128, :], out_sb[:])

    split_multi_waits(nc)
    return nc


_NC_CACHE = {}
LAST_RESULTS = None


def _gumbel_full():
    import jax

    cpu = jax.devices("cpu")[0]
    with jax.default_device(cpu):
        g = jax.random.gumbel(jax.random.key(42), (G, D), "float32")
        return np.asarray(g)


def kernel(
    raw_Z, gen_Z, W1z, b1z, W2z, b2z, W1g, b1g, W2g, b2g, G_rep, **kw
) -> np.ndarray:
    global LAST_RESULTS
    if "nc" not in _NC_CACHE:
        _NC_CACHE["nc"] = build_nc()
    nc = _NC_CACHE["nc"]

    gumb = _gumbel_full()
    raw_Z = np.ascontiguousarray(np.asarray(raw_Z, np.float32))
    W1z = np.ascontiguousarray(np.asarray(W1z, np.float32))
    shared = {
        "b1z": np.asarray(b1z, np.float32),
        "W2z": np.ascontiguousarray(np.asarray(W2z, np.float32)),
        "b2z": np.asarray(b2z, np.float32),
        "W1g": np.ascontiguousarray(np.asarray(W1g, np.float32)),
        "b1g": np.asarray(b1g, np.float32),
        "W2g": np.ascontiguousarray(np.asarray(W2g, np.float32)),
        "b2g": np.asarray(b2g, np.float32),
        "gen_Z": np.ascontiguousarray(np.asarray(gen_Z, np.float32)),
    }
    G_rep = np.asarray(G_rep, np.float32)
    in_maps = []
    for c in range(N_CORES):
        m = dict(shared)
        if SHARD_A:
            m["raw_Z_s"] = np.ascontiguousarray(raw_Z[c * CS : (c + 1) * CS])
            m["W1z_s"] = np.ascontiguousarray(W1z[c * CS : (c + 1) * CS])
        else:
            m["raw_Z_s"] = raw_Z
            m["W1z_s"] = W1z
        m["G_rep_s"] = np.ascontiguousarray(G_rep[c * GS : (c + 1) * GS])
        m["gumb_s"] = np.ascontiguousarray(gumb[c * GS : (c + 1) * GS])
        in_maps.append(m)

    res = run_bass_kernel_spmd(nc, in_maps, core_ids=list(range(N_CORES)))
    LAST_RESULTS = res
    out = np.concatenate([res.results[c]["out_s"] for c in range(N_CORES)], axis=1)
    return out
